# revision 15
# baseline (speedup 1.0000x reference)
"""Trainium2 Bass kernel for a single nGPT-style attention head.

Computation (see reference): fused QKV projection, RoPE over the full head
dim, L2-normalize q/k scaled by sqk, causal SDPA with scale sqrt(d_model).

Sharding: data-parallel over batch — 8 batch elements, one per NeuronCore.
Each core gets x[b] (pre-transposed on host to [C, T] so the contraction
dim lands on SBUF partitions), the shared QKV weight (pre-transposed to
[C, 3D]), precomputed RoPE cos/sin tables, a causal triangle mask tile and
sqk. The core computes out^T = [D, T]; the host transposes back and stacks.

Device-side layout choices:
  - qkv^T is computed as [d, t] (d on partitions) so q^T/k^T are directly
    usable as matmul operands for scores^T = k_tile^T-as-lhsT @ q-block.
  - scores are computed transposed: S^T[tk, tq] per (tk-tile=128, tq-block
    =512). exp runs on ACT reading PSUM; softmax denominator comes from a
    ones-vector matmul over the exp strips; attn@v accumulates
    out^T[e, tq] with v tiles [tk, e] as stationary operands (v transposed
    on-chip via DMA-transpose, bf16).
  - Normalization: ||q|| is rotation-invariant, so norms are computed from
    pre-RoPE q/k via ACT Square + ones-matmul partition reduction; the
    1/norm row is broadcast across partitions with a rank-1 matmul.
  - dtypes: QKV projection in float32r (full-rate fp32 mode); attention
    matmuls in bf16 with fp32 PSUM accumulation; final softmax division
    in fp32.
"""

import math

import numpy as np
import ml_dtypes

import concourse.bass as bass
import concourse.tile as tile
from concourse import bacc, mybir
from concourse.bass import ts, ds
from concourse.bass_utils import run_bass_kernel_spmd

# Surface compile-hook exceptions (the PJRT bridge swallows tracebacks).
try:
    import traceback
    import libneuronxla as _lnx

    if not getattr(_lnx, "_err_wrapped", False):
        _orig_cc = _lnx.neuronx_cc

        def _cc_wrapper(*a, **kw):
            try:
                return _orig_cc(*a, **kw)
            except BaseException:
                traceback.print_exc()
                raise

        _lnx.neuronx_cc = _cc_wrapper
        _lnx._err_wrapped = True
except Exception:
    pass

AFT = mybir.ActivationFunctionType
F32 = mybir.dt.float32
F32R = mybir.dt.float32r
BF16 = mybir.dt.bfloat16

B, T_FULL, C, D = 8, 2048, 1024, 128
ROPE_BASE = 10000.0
P = 128
TB = 512  # t-block (tq block width, PSUM-bank free dim)
NCO = C // P  # contraction chunks for the QKV projection


def build_nc(T=T_FULL, num_devices=8):
    NTB = T // TB
    NKT = T // P
    nc = bacc.Bacc("TRN2", target_bir_lowering=False, debug=False,
                   num_devices=num_devices)

    xT = nc.dram_tensor("xT", [C, T], F32R, kind="ExternalInput").ap()
    WT = nc.dram_tensor("WT", [C, 3 * D], F32R, kind="ExternalInput").ap()
    cosF = nc.dram_tensor("cosF", [P, 2 * T], BF16, kind="ExternalInput").ap()
    sinF = nc.dram_tensor("sinF", [P, 2 * T], BF16, kind="ExternalInput").ap()
    tri = nc.dram_tensor("tri", [P, P], BF16, kind="ExternalInput").ap()
    sqk = nc.dram_tensor("sqk", [D, 1], F32, kind="ExternalInput").ap()
    onr = nc.dram_tensor("onr", [P, 1], F32R, kind="ExternalInput").ap()
    zro = nc.dram_tensor("zro", [P, 3 * P], BF16, kind="ExternalInput").ap()
    onb = nc.dram_tensor("onb", [P, 1], BF16, kind="ExternalInput").ap()
    on1 = nc.dram_tensor("on1", [1, P], F32R, kind="ExternalInput").ap()
    outT = nc.dram_tensor("outT", [D, T], F32, kind="ExternalOutput").ap()

    xT_t = xT.rearrange("(co p) t -> p co t", p=P)
    WT_t = WT.rearrange("(co p) d -> p co d", p=P)

    with tile.TileContext(nc) as tc:
        from contextlib import ExitStack
        with ExitStack() as ctx:
            const = ctx.enter_context(tc.tile_pool(name="const", bufs=1))

            wt = const.tile([P, NCO, 3 * D], F32R)
            nc.sync.dma_start(wt, WT_t)
            cos_sb = const.tile([P, 2 * T], BF16)
            nc.sync.dma_start(cos_sb, cosF)
            sin_sb = const.tile([P, 2 * T], BF16)
            nc.sync.dma_start(sin_sb, sinF)
            tri_sb = const.tile([P, P], BF16)
            nc.sync.dma_start(tri_sb, tri)
            sqk_sb = const.tile([D, 1], F32)
            nc.sync.dma_start(sqk_sb, sqk)
            # (sqk * C^(1/4))^2 = sqrt(C) * sqk^2 — the full logit scale,
            # folded into q.
            sqk232 = const.tile([D, 1], F32)
            nc.scalar.activation(sqk232, sqk_sb, AFT.Square, scale=float(C ** 0.25))

            ones_d = const.tile([P, 1], F32R)
            nc.sync.dma_start(ones_d, onr)
            ones_k = const.tile([P, 1], BF16)
            nc.sync.dma_start(ones_k, onb)
            ones_1 = const.tile([1, P], F32R)
            nc.sync.dma_start(ones_1, on1)

            # persistent activations
            qk = const.tile([P, 2 * T], BF16)    # q̃^T | k̃^T (post everything)
            vst = const.tile([P, T], BF16)       # v^T staging
            vt = const.tile([P, NKT, P], BF16)   # v tiles [tk, e]
            nrm = const.tile([1, 2 * T], F32)    # ||q|| , ||k||
            invn = const.tile([1, 2 * T], F32R)  # 1/||q|| , 1/||k||

            # ---------------- Phase A: QKV projection ----------------
            with ExitStack() as actx:
                apool = actx.enter_context(tc.tile_pool(name="apool", bufs=2))
                sqp = actx.enter_context(tc.tile_pool(name="sqp", bufs=2))
                ps_qkv = actx.enter_context(
                    tc.tile_pool(name="ps_qkv", bufs=3, space="PSUM"))
                ps_n = actx.enter_context(
                    tc.tile_pool(name="ps_n", bufs=2, space="PSUM"))
                for j in range(NTB):
                    xt = apool.tile([P, NCO, TB], F32R, tag="xt")
                    nc.sync.dma_start(xt, xT_t[:, :, ts(j, TB)])
                    for g in range(3):
                        ps = ps_qkv.tile([P, TB], F32, tag="qkv")
                        for co in range(NCO):
                            nc.tensor.matmul(
                                ps, wt[:, co, ts(g, D)], xt[:, co, :],
                                start=(co == 0), stop=(co == NCO - 1))
                        if g < 2:
                            sq = sqp.tile([P, TB], F32R, tag="sq")
                            nc.scalar.activation(sq, ps, AFT.Square)
                            nps = ps_n.tile([1, TB], F32, tag="n")
                            nc.tensor.matmul(nps, ones_d, sq,
                                             start=True, stop=True)
                            nc.scalar.activation(
                                nrm[:, ds(g * T + j * TB, TB)], nps, AFT.Sqrt)
                            nc.any.tensor_copy(
                                out=qk[:, ds(g * T + j * TB, TB)], in_=ps)
                        else:
                            nc.any.tensor_copy(
                                out=vst[:, ds(j * TB, TB)], in_=ps)

            # ---------------- Phase B: RoPE + normalize + v transpose ----
            with ExitStack() as bctx:
                bpool = bctx.enter_context(tc.tile_pool(name="bpool", bufs=1))
                dramp = bctx.enter_context(
                    tc.tile_pool(name="dramp", bufs=1, space="DRAM"))
                ps_b = bctx.enter_context(
                    tc.tile_pool(name="ps_b", bufs=2, space="PSUM"))

                H = P // 2
                rot = bpool.tile([P, 2 * T], BF16)
                nc.vector.tensor_scalar_mul(rot[0:H, :], qk[H:P, :], -1.0)
                nc.vector.tensor_copy(rot[H:P, :], qk[0:H, :])
                t1 = bpool.tile([P, 2 * T], BF16)
                nc.vector.tensor_mul(t1, qk, cos_sb)
                t2 = bpool.tile([P, 2 * T], BF16)
                nc.vector.tensor_mul(t2, rot, sin_sb)
                nc.vector.tensor_add(t1, t1, t2)  # t1 = rope(qk)

                with nc.allow_low_precision(reason="f32r view for matmul"):
                    nc.vector.reciprocal(invn, nrm)

                for m in range(2 * NTB):
                    bc = ps_b.tile([P, TB], F32, tag="bc")
                    nc.tensor.matmul(bc, ones_1, invn[:, ts(m, TB)],
                                     start=True, stop=True)
                    if m < NTB:  # q chunk: fold sqrt(C)*sqk^2
                        nc.vector.scalar_tensor_tensor(
                            out=qk[:, ts(m, TB)], in0=t1[:, ts(m, TB)],
                            scalar=sqk232, in1=bc,
                            op0=mybir.AluOpType.mult, op1=mybir.AluOpType.mult)
                    else:
                        nc.vector.tensor_mul(qk[:, ts(m, TB)],
                                             t1[:, ts(m, TB)], bc)

                # v transpose via DMA xbar (bf16): [d, t]-tile -> [t, d]-tile.
                # Round-trips through DRAM: the direct SBUF->SBUF transpose
                # DMA races ahead of the copies producing vst (scheduler
                # misses the dependency) and DRAM->SBUF is the proven path.
                vd = dramp.tile([P, T], BF16)
                nc.sync.dma_start(vd, vst)
                for i in range(NKT):
                    nc.sync.dma_start_transpose(vt[:, i, :], vd[:, ts(i, P)])

            # ---------------- Phase C: causal attention ----------------
            with ExitStack() as cctx:
                expool = cctx.enter_context(tc.tile_pool(name="expool", bufs=3))
                dpool = cctx.enter_context(tc.tile_pool(name="dpool", bufs=2))
                ps_sc = cctx.enter_context(
                    tc.tile_pool(name="ps_sc", bufs=2, space="PSUM"))
                ps_o = cctx.enter_context(
                    tc.tile_pool(name="ps_o", bufs=2, space="PSUM"))
                ps_d = cctx.enter_context(
                    tc.tile_pool(name="ps_d", bufs=2, space="PSUM"))

                for J in range(NTB):
                    q_blk = qk[:, ts(J, TB)]
                    po = ps_o.tile([P, TB], F32, tag="o")
                    pd = ps_d.tile([1, TB], F32, tag="d")
                    nstr = (TB // P) * (J + 1)
                    for g in range(nstr // 2):
                        strips = [2 * g, 2 * g + 1]
                        sc = ps_sc.tile([P, 2, TB], F32, tag="sc")
                        ex = expool.tile([P, 2, TB], BF16, tag="ex")
                        offs = []
                        for r2, i in enumerate(strips):
                            dr = i - (TB // P) * J  # >=0 on diagonal strips
                            off = P * dr if dr >= 0 else 0
                            offs.append(off)
                            # scores^T strip: [tk=128, tq=TB-off]
                            nc.tensor.matmul(
                                sc[:, r2, ds(off, TB - off)],
                                qk[:, ds(T + P * i, P)],
                                q_blk[:, ds(off, TB - off)],
                                start=True, stop=True)
                        if offs == [0, 0]:
                            # both strips full: one batched exp over 2 banks
                            nc.scalar.activation(ex, sc, AFT.Exp)
                        else:
                            for r2, i in enumerate(strips):
                                off = offs[r2]
                                nc.scalar.activation(
                                    ex[:, r2, ds(off, TB - off)],
                                    sc[:, r2, ds(off, TB - off)], AFT.Exp)
                        for r2, i in enumerate(strips):
                            off = offs[r2]
                            if i - (TB // P) * J >= 0:
                                # triangle mask on the diagonal 128x128 square
                                nc.vector.tensor_mul(
                                    ex[:, r2, ds(off, P)],
                                    ex[:, r2, ds(off, P)], tri_sb)
                                if off > 0:
                                    nc.sync.dma_start(ex[:, r2, ds(0, off)],
                                                      zro[:, ds(0, off)])
                        for r2, i in enumerate(strips):
                            nc.tensor.matmul(po, vt[:, i, :], ex[:, r2, :],
                                             start=(i == 0), stop=(i == nstr - 1))
                            nc.tensor.matmul(pd, ones_k, ex[:, r2, :],
                                             start=(i == 0), stop=(i == nstr - 1))

                    invd = dpool.tile([1, TB], F32R, tag="invd")
                    with nc.allow_low_precision(reason="f32r view for matmul"):
                        nc.vector.reciprocal(invd, pd)
                    bc2 = ps_sc.tile([P, TB], F32, tag="sc")
                    nc.tensor.matmul(bc2, ones_1, invd, start=True, stop=True)
                    bc2s = dpool.tile([P, TB], F32, tag="bc2s")
                    nc.any.tensor_copy(out=bc2s, in_=bc2)
                    ob = dpool.tile([P, TB], F32, tag="ob")
                    nc.vector.tensor_mul(ob, po, bc2s)
                    nc.sync.dma_start(outT[:, ts(J, TB)], ob)

    nc.compile()
    return nc


def _host_tables(T):
    d = D
    inv_freq = 1.0 / (ROPE_BASE ** (np.arange(0, d, 2, dtype=np.float64) / d))
    t = np.arange(T, dtype=np.float64)
    freqs = np.outer(inv_freq, t)  # [d/2, T]
    emb = np.concatenate([freqs, freqs], axis=0)  # [d, T]
    cos1 = np.cos(emb)
    sin1 = np.sin(emb)
    cosF = np.concatenate([cos1, cos1], axis=1).astype(ml_dtypes.bfloat16)
    sinF = np.concatenate([sin1, sin1], axis=1).astype(ml_dtypes.bfloat16)
    a = np.arange(P)
    tri = (a[None, :] >= a[:, None]).astype(ml_dtypes.bfloat16)  # [tk, tq]
    return cosF, sinF, tri


TRACE = False
LAST_EXEC_NS = None
LAST_TRACE = None
LAST_INSTS = None


def kernel(x, W_qkv, sqk):
    global LAST_EXEC_NS, LAST_TRACE
    T = x.shape[1]
    cosF, sinF, tri = _host_tables(T)
    WT = np.ascontiguousarray(W_qkv.T).astype(np.float32)
    sqk2 = np.ascontiguousarray(sqk.reshape(D, 1)).astype(np.float32)
    in_maps = []
    for b in range(B):
        in_maps.append({
            "xT": np.ascontiguousarray(np.asarray(x[b]).T).astype(np.float32),
            "WT": WT,
            "cosF": cosF,
            "sinF": sinF,
            "tri": tri,
            "sqk": sqk2,
            "onr": np.ones((P, 1), np.float32),
            "zro": np.zeros((P, 3 * P), ml_dtypes.bfloat16),
            "onb": np.ones((P, 1), ml_dtypes.bfloat16),
            "on1": np.ones((1, P), np.float32),
        })
    nc = build_nc(T=T, num_devices=B)
    res = run_bass_kernel_spmd(nc, in_maps, core_ids=list(range(B)),
                               trace=TRACE)
    LAST_EXEC_NS = res.exec_time_ns
    LAST_TRACE = (res.instructions_and_trace[1]
                  if res.instructions_and_trace else None)
    global LAST_INSTS
    LAST_INSTS = (res.instructions_and_trace[0]
                  if res.instructions_and_trace else None)
    out = np.stack([r["outT"].T for r in res.results])  # [B, T, D]
    return np.ascontiguousarray(out).astype(np.float32)
